# revision 4
# baseline (speedup 1.0000x reference)
"""GRU decoder Trainium2 kernel v2 (data-parallel over batch, 8 cores).

Reference (per step s, gate order r,z,n):
    gi = x_s @ w_ih.T + b_ih ; gh = h_s @ w_hh.T + b_hh
    r = sig(...); z = sig(...); n = tanh(i_n + r*h_n)
    h_{s+1} = (1-z)*n + z*h_s ; y_s = h_{s+1} @ w_fc.T + b_fc ; x_{s+1} = y_s

Key algebraic restructuring: for s >= 1, x_s = y_{s-1} = h_s @ w_fc.T + b_fc,
so  gi_s = h_s @ W_c + bc  with  W_c = w_fc.T @ w_ih.T,  bc = b_ih + b_fc@w_ih.T.
Both gi_s and gh_s are then linear in h_s, and the r/z parts share one matrix:
    grz = h_s @ W_rz + brz,  W_rz = (W_c + w_hh.T)[:, :2H],
    i_n = h_s @ W_c[:, 2H:],  h_n = h_s @ w_hh.T[:, 2H:].
The fc layer (y) disappears from the recurrence critical path entirely; y_s is
computed from h_{s+1}^T after the next step's gate matmuls have been issued.
Step 0 (which consumes src[0]) runs on the host; the device runs steps 1..255.

Per-core layout (batch 32): a [128, C] tile packs rows 32*j+b (feature-chunk j,
batch b).  Gate matmuls keep batch in the PE stationary free dim (M=32) with 4
column-groups via tile_position=(0,32j), each handling 256 hidden units.
Biases enter PSUM via a K=4 matmul (lhsT=ONE4) that also opens the
accumulation group, so gi_n|h_n can share a single PSUM bank.

Scheduling within a step: all rz matmuls stream before the i_n|h_n matmuls so
the sigmoids (and zh = z*h, 1-z) hide behind the second half of the gate
stream; the post-matmul chain (rt, ns, tanh, h') is split into free-dim halves
so the first h'-transpose (which feeds the next step's first 4 gate chunks)
fires as early as possible.  h lives as two half tiles Hp0|Hp1 to keep the
dependency tracking per-half.
"""

import numpy as np
import ml_dtypes

import concourse.bass as bass
import concourse.bacc as bacc
import concourse.tile as tile
from concourse import mybir
from concourse.bass_utils import run_bass_kernel_spmd

H = 1024
O = 768
B = 256
T = 256
NCORES = 8
BC = B // NCORES  # 32 batch rows per core
KH = H // 128  # 8 h-contraction chunks
NSTEPS = T - 1  # device steps (step 0 on host)

# weight tile column offsets inside WB (bf16)
WRZ_OFF = 0
WNIH_OFF = KH * 4 * 512            # 16384
WFY_OFF = WNIH_OFF + KH * 4 * 512  # 32768
NB = WFY_OFF + KH * 4 * 192        # 38912

F32 = mybir.dt.float32
BF16 = mybir.dt.bfloat16
AF = mybir.ActivationFunctionType
ALU = mybir.AluOpType

_COMPILED = {}

KORDER = [0, 2, 4, 6, 1, 3, 5, 7]  # hsb0 chunks first


def _build_nc(nsteps=NSTEPS, yo_len=None):
    nc = bacc.Bacc("TRN2", target_bir_lowering=False, debug=False, num_devices=NCORES)

    wb = nc.declare_dram_parameter("WB", [128, NB], BF16, isOutput=False)
    cf = nc.declare_dram_parameter("CF", [128, 1408], F32, isOutput=False)
    yo = nc.declare_dram_parameter(
        "YO", [yo_len or nsteps, 128, 192], F32, isOutput=True
    )

    def wrz(k, j):
        off = WRZ_OFF + (k * 4 + j) * 512
        return WB_t[:, off : off + 512]

    def wnih(k, j):
        off = WNIH_OFF + (k * 4 + j) * 512
        return WB_t[:, off : off + 512]

    def wfy(k, j):
        off = WFY_OFF + (k * 4 + j) * 192
        return WB_t[:, off : off + 192]

    with tile.TileContext(nc) as tc:
        with (
            tc.tile_pool(name="wpool", bufs=1) as wpool,
            tc.tile_pool(name="state", bufs=2) as spool,
            tc.tile_pool(name="act", bufs=2) as apool,
            tc.tile_pool(name="psG", bufs=1, space="PSUM") as psG,
            tc.tile_pool(name="psT", bufs=1, space="PSUM") as psT,
            tc.tile_pool(name="psY", bufs=2, space="PSUM") as psY,
        ):
            WB_t = wpool.tile([128, NB], BF16, tag="WB")
            CF_t = wpool.tile([128, 1408], F32, tag="CF")
            nc.sync.dma_start(WB_t[:], wb[:])
            nc.sync.dma_start(CF_t[:], cf[:])

            IDT = CF_t[:, 0:128]
            BRZ = CF_t[:, 384:896]
            BNH = CF_t[:, 896:1152]
            BNI = CF_t[:, 1152:1408]

            # ---- initial state: Hp0|Hp1 = h_1 packed, hsb = h_1^T (bf16) ----
            Hp0 = spool.tile([128, 128], F32, tag="Hp0")
            Hp1 = spool.tile([128, 128], F32, tag="Hp1")
            nc.vector.tensor_copy(Hp0[:], CF_t[:, 128:256])
            nc.vector.tensor_copy(Hp1[:], CF_t[:, 256:384])

            def transpose_half(src_tile, tag_ps, tag_sb, engine):
                tp = psT.tile([128, 128], F32, tag=tag_ps)
                nc.tensor.transpose(tp[:], src_tile[:], IDT)
                h = spool.tile([128, 128], BF16, tag=tag_sb)
                if engine == "v":
                    nc.vector.tensor_copy(h[:], tp[:])
                else:
                    nc.scalar.activation(h[:], tp[:], AF.Copy)
                return h

            hsb0 = transpose_half(Hp0, "tp0", "hsb0", "v")
            hsb1 = transpose_half(Hp1, "tp1", "hsb1", "s")

            def y_block(h0, h1):
                """fc(h') from transposed state tiles -> psum tile."""
                yps = psY.tile([128, 192], F32, tag="yps")
                for k in range(KH):
                    src = h0 if k % 2 == 0 else h1
                    lhsT = src[:, 32 * (k // 2) : 32 * (k // 2) + 32]
                    for j in range(4):
                        nc.tensor.matmul(
                            yps[32 * j : 32 * j + 32, :],
                            lhsT,
                            wfy(k, j),
                            start=(k == 0),
                            stop=(k == KH - 1),
                            tile_position=(0, 32 * j),
                            skip_group_check=True,
                        )
                return yps

            for i in range(nsteps):
                # ---------- gate matmuls (consume hsb = h_{i+1}^T) ----------
                # three banks, streamed rz -> h_n -> i_n so that the sigmoids,
                # bias adds and rt = r*(h_n+bnh) hide behind the later passes.
                # The i_n half-split lets the half-0 chain (ns0/tanh0/h'0) run
                # while the second i_n half and the y matmuls still stream.
                grz = psG.tile([128, 512], F32, tag="grz")
                ghn = psG.tile([128, 256], F32, tag="ghn")
                gin = psG.tile([128, 256], F32, tag="gin")
                for base, lo, width, colo, gout, closes in (
                    (WRZ_OFF, 0, 512, 0, grz, True),
                    (WNIH_OFF, 256, 256, 0, ghn, True),
                    (WNIH_OFF, 0, 128, 0, gin, False),
                    (WNIH_OFF, 128, 128, 128, gin, True),
                ):
                    for k in KORDER:
                        src = hsb0 if k % 2 == 0 else hsb1
                        lhsT = src[:, 32 * (k // 2) : 32 * (k // 2) + 32]
                        first = k == KORDER[0]
                        last = k == KORDER[-1] and closes
                        for j in range(4):
                            off = base + (k * 4 + j) * 512 + lo
                            nc.tensor.matmul(
                                gout[32 * j : 32 * j + 32, colo : colo + width],
                                lhsT,
                                WB_t[:, off : off + width],
                                start=first,
                                stop=last,
                                tile_position=(0, 32 * j),
                                skip_group_check=True,
                            )

                phsb0, phsb1 = hsb0, hsb1  # y_i reads these after the transposes

                # ---------- elementwise: h_{i+2} = (1-z)*n + z*h_{i+1} ------
                # bias adds + sigmoids + zh + b1z hide behind the matmul
                # stream; zh/b1z run on the (otherwise idle) gpsimd engine
                trz = apool.tile([128, 512], F32, tag="trz")
                nc.vector.tensor_tensor(trz[:, 0:256], grz[:, 0:256], BRZ[:, 0:256], ALU.add)
                r = apool.tile([128, 256], F32, tag="r")
                nc.scalar.activation(r[:], trz[:, 0:256], AF.Sigmoid)
                nc.vector.tensor_tensor(trz[:, 256:512], grz[:, 256:512], BRZ[:, 256:512], ALU.add)
                z = apool.tile([128, 256], F32, tag="z")
                nc.scalar.activation(z[:], trz[:, 256:512], AF.Sigmoid)
                zh = apool.tile([128, 256], F32, tag="zh")
                nc.gpsimd.tensor_tensor(zh[:, 0:128], z[:, 0:128], Hp0[:], ALU.mult)
                nc.gpsimd.tensor_tensor(zh[:, 128:256], z[:, 128:256], Hp1[:], ALU.mult)
                b1z = apool.tile([128, 256], F32, tag="b1z")
                nc.gpsimd.tensor_scalar(b1z[:], z[:], -1.0, 1.0, ALU.mult, ALU.add)

                # hnb/rt/rtb run while the i_n pass still streams
                hnb = apool.tile([128, 256], F32, tag="hnb")
                nc.vector.tensor_tensor(hnb[:], ghn[:], BNH[:], ALU.add)
                rt = apool.tile([128, 256], F32, tag="rt")
                nc.vector.tensor_tensor(rt[:], r[:], hnb[:], ALU.mult)
                rtb = apool.tile([128, 256], F32, tag="rtb")
                nc.vector.tensor_tensor(rtb[:], rt[:], BNI[:], ALU.add)

                # chain, half 0 first (feeds hsb0 -> next step's first chunks)
                ns = apool.tile([128, 256], F32, tag="ns")
                n = apool.tile([128, 256], F32, tag="n")
                Hp0n = spool.tile([128, 128], F32, tag="Hp0")
                Hp1n = spool.tile([128, 128], F32, tag="Hp1")
                for h0c, h1c, Hpn in ((0, 128, Hp0n), (128, 256, Hp1n)):
                    nc.vector.tensor_tensor(
                        ns[:, h0c:h1c], rtb[:, h0c:h1c], gin[:, h0c:h1c], ALU.add
                    )
                    nc.scalar.activation(n[:, h0c:h1c], ns[:, h0c:h1c], AF.Tanh)
                    nc.vector.tensor_tensor(
                        Hpn[:], n[:, h0c:h1c], b1z[:, h0c:h1c], ALU.mult
                    )
                    nc.vector.tensor_tensor(Hpn[:], Hpn[:], zh[:, h0c:h1c], ALU.add)
                Hp0, Hp1 = Hp0n, Hp1n

                # ---------- transposes for next step, then y matmuls --------
                # y (which only needs the OLD hsb) streams right after the
                # transposes, covering the PE wait for the hsb copies.
                hsb0 = transpose_half(Hp0, "tp0", "hsb0", "v")
                hsb1 = transpose_half(Hp1, "tp1", "hsb1", "s")
                if i > 0:
                    yps = y_block(phsb0, phsb1)
                    ysb = apool.tile([128, 192], F32, tag="ysb")
                    nc.scalar.activation(ysb[:], yps[:], AF.Copy)
                    nc.sync.dma_start(yo[i - 1], ysb[:])

            # ---------- drain final y (y_255 from h_256^T) ----------
            yps = y_block(hsb0, hsb1)
            ysb = apool.tile([128, 192], F32, tag="ysb")
            nc.scalar.activation(ysb[:], yps[:], AF.Copy)
            nc.sync.dma_start(yo[nsteps - 1], ysb[:])

    nc.compile()
    return nc


def _sigmoid(x):
    return 1.0 / (1.0 + np.exp(-x))


def _host_step0(src, hidden, w_ih, w_hh, b_ih, b_hh, w_fc, b_fc):
    """Reference step 0 in numpy f32: returns (h_1 [B,H], y_0 [B,O])."""
    x0 = src[0]
    h0 = hidden[0]
    gi = x0 @ w_ih.T + b_ih
    gh = h0 @ w_hh.T + b_hh
    r = _sigmoid(gi[:, 0:H] + gh[:, 0:H])
    z = _sigmoid(gi[:, H : 2 * H] + gh[:, H : 2 * H])
    n = np.tanh(gi[:, 2 * H :] + r * gh[:, 2 * H :])
    h1 = (1.0 - z) * n + z * h0
    y0 = h1 @ w_fc.T + b_fc
    return h1.astype(np.float32), y0.astype(np.float32)


def _prep_shared(w_ih, w_hh, b_ih, b_hh, w_fc, b_fc):
    """Host-side weight/bias packing shared by all cores -> (WB, CB)."""
    W_c = w_fc.T @ w_ih.T  # [H, 3H]
    Whh_T = np.ascontiguousarray(w_hh.T)  # [H, 3H]
    bc = b_ih + b_fc @ w_ih.T  # [3H]

    W_rz = W_c[:, : 2 * H] + Whh_T[:, : 2 * H]
    brz = bc[: 2 * H] + b_hh[: 2 * H]
    WNI = W_c[:, 2 * H :]
    WNH = Whh_T[:, 2 * H :]
    bni = bc[2 * H :]
    bnh = b_hh[2 * H :]

    wb = np.empty((128, NB), np.float32)
    for k in range(KH):
        rz_rows = W_rz[128 * k : 128 * k + 128]  # [128, 2048]
        ni_rows = WNI[128 * k : 128 * k + 128]  # [128, 1024]
        nh_rows = WNH[128 * k : 128 * k + 128]
        fy_rows = w_fc.T[128 * k : 128 * k + 128]  # [128, 768]
        for j in range(4):
            off = WRZ_OFF + (k * 4 + j) * 512
            wb[:, off : off + 256] = rz_rows[:, 256 * j : 256 * j + 256]
            wb[:, off + 256 : off + 512] = rz_rows[:, H + 256 * j : H + 256 * j + 256]
            off = WNIH_OFF + (k * 4 + j) * 512
            wb[:, off : off + 256] = ni_rows[:, 256 * j : 256 * j + 256]
            wb[:, off + 256 : off + 512] = nh_rows[:, 256 * j : 256 * j + 256]
            off = WFY_OFF + (k * 4 + j) * 192
            wb[:, off : off + 192] = fy_rows[:, 192 * j : 192 * j + 192]
    WB = wb.astype(ml_dtypes.bfloat16)

    def pack_bias(vec):  # [1024] -> [128, 256], rows 32j+b get slice j
        return np.repeat(vec.reshape(4, 256), 32, axis=0).astype(np.float32)

    biases = np.concatenate(
        [pack_bias(brz[:H]), pack_bias(brz[H:]), pack_bias(bnh), pack_bias(bni)],
        axis=1,
    )  # [128, 1024] = BRZ(512) | BNH(256) | BNI(256)
    return WB, biases


def _prep_core(h1, biases):
    """h1 [32, 1024] -> CF [128, 1408] f32 (IDT | HPI | BRZ | BNH | BNI)."""
    cfm = np.zeros((128, 1408), np.float32)
    cfm[:, 0:128] = np.eye(128, dtype=np.float32)
    cfm[:, 128:384] = h1.reshape(BC, 4, 256).transpose(1, 0, 2).reshape(128, 256)
    cfm[:, 384:1408] = biases
    return cfm


def kernel(src, tgt, hidden, w_ih, w_hh, b_ih, b_hh, w_fc, b_fc, **_kw):
    src = np.asarray(src, np.float32)
    hidden = np.asarray(hidden, np.float32)
    w_ih = np.asarray(w_ih, np.float32)
    w_hh = np.asarray(w_hh, np.float32)
    b_ih = np.asarray(b_ih, np.float32)
    b_hh = np.asarray(b_hh, np.float32)
    w_fc = np.asarray(w_fc, np.float32)
    b_fc = np.asarray(b_fc, np.float32)

    WB, biases = _prep_shared(w_ih, w_hh, b_ih, b_hh, w_fc, b_fc)
    h1, y0 = _host_step0(src, hidden, w_ih, w_hh, b_ih, b_hh, w_fc, b_fc)

    if NSTEPS not in _COMPILED:
        _COMPILED[NSTEPS] = _build_nc(NSTEPS)
    nc = _COMPILED[NSTEPS]

    in_maps = []
    for c in range(NCORES):
        sl = slice(BC * c, BC * (c + 1))
        in_maps.append(dict(WB=WB, CF=_prep_core(h1[sl], biases)))

    res = run_bass_kernel_spmd(nc, in_maps, list(range(NCORES)))

    out = np.empty((T, B, O), np.float32)
    out[0] = y0
    for c in range(NCORES):
        sl = slice(BC * c, BC * (c + 1))
        yo = np.asarray(res.results[c]["YO"])  # [255, 128, 192]
        out[1:, sl] = (
            yo.reshape(NSTEPS, 4, BC, 192).transpose(0, 2, 1, 3).reshape(NSTEPS, BC, O)
        )
    out[1:] += b_fc[None, None, :]
    return out


# revision 5
# speedup vs baseline: 1.0002x; 1.0002x over previous
"""GRU decoder Trainium2 kernel v2 (data-parallel over batch, 8 cores).

Reference (per step s, gate order r,z,n):
    gi = x_s @ w_ih.T + b_ih ; gh = h_s @ w_hh.T + b_hh
    r = sig(...); z = sig(...); n = tanh(i_n + r*h_n)
    h_{s+1} = (1-z)*n + z*h_s ; y_s = h_{s+1} @ w_fc.T + b_fc ; x_{s+1} = y_s

Key algebraic restructuring: for s >= 1, x_s = y_{s-1} = h_s @ w_fc.T + b_fc,
so  gi_s = h_s @ W_c + bc  with  W_c = w_fc.T @ w_ih.T,  bc = b_ih + b_fc@w_ih.T.
Both gi_s and gh_s are then linear in h_s, and the r/z parts share one matrix:
    grz = h_s @ W_rz + brz,  W_rz = (W_c + w_hh.T)[:, :2H],
    i_n = h_s @ W_c[:, 2H:],  h_n = h_s @ w_hh.T[:, 2H:].
The fc layer (y) disappears from the recurrence critical path entirely; y_s is
computed from h_{s+1}^T after the next step's gate matmuls have been issued.
Step 0 (which consumes src[0]) runs on the host; the device runs steps 1..255.

Per-core layout (batch 32): a [128, C] tile packs rows 32*j+b (feature-chunk j,
batch b).  Gate matmuls keep batch in the PE stationary free dim (M=32) with 4
column-groups via tile_position=(0,32j), each handling 256 hidden units.
Biases enter PSUM via a K=4 matmul (lhsT=ONE4) that also opens the
accumulation group, so gi_n|h_n can share a single PSUM bank.

Scheduling within a step: all rz matmuls stream before the i_n|h_n matmuls so
the sigmoids (and zh = z*h, 1-z) hide behind the second half of the gate
stream; the post-matmul chain (rt, ns, tanh, h') is split into free-dim halves
so the first h'-transpose (which feeds the next step's first 4 gate chunks)
fires as early as possible.  h lives as two half tiles Hp0|Hp1 to keep the
dependency tracking per-half.
"""

import numpy as np
import ml_dtypes

import concourse.bass as bass
import concourse.bacc as bacc
import concourse.tile as tile
from concourse import mybir
from concourse.bass_utils import run_bass_kernel_spmd

H = 1024
O = 768
B = 256
T = 256
NCORES = 8
BC = B // NCORES  # 32 batch rows per core
KH = H // 128  # 8 h-contraction chunks
NSTEPS = T - 1  # device steps (step 0 on host)

# weight tile column offsets inside WB (bf16)
WRZ_OFF = 0
WNIH_OFF = KH * 4 * 512            # 16384
WFY_OFF = WNIH_OFF + KH * 4 * 512  # 32768
NB = WFY_OFF + KH * 4 * 192        # 38912

F32 = mybir.dt.float32
BF16 = mybir.dt.bfloat16
AF = mybir.ActivationFunctionType
ALU = mybir.AluOpType

_COMPILED = {}

KORDER = [0, 2, 4, 6, 1, 3, 5, 7]  # hsb0 chunks first


def _build_nc(nsteps=NSTEPS, yo_len=None):
    nc = bacc.Bacc("TRN2", target_bir_lowering=False, debug=False, num_devices=NCORES)

    wb = nc.declare_dram_parameter("WB", [128, NB], BF16, isOutput=False)
    cf = nc.declare_dram_parameter("CF", [128, 1408], F32, isOutput=False)
    yo = nc.declare_dram_parameter(
        "YO", [yo_len or nsteps, 128, 192], F32, isOutput=True
    )

    def wrz(k, j):
        off = WRZ_OFF + (k * 4 + j) * 512
        return WB_t[:, off : off + 512]

    def wnih(k, j):
        off = WNIH_OFF + (k * 4 + j) * 512
        return WB_t[:, off : off + 512]

    def wfy(k, j):
        off = WFY_OFF + (k * 4 + j) * 192
        return WB_t[:, off : off + 192]

    with tile.TileContext(nc) as tc:
        with (
            tc.tile_pool(name="wpool", bufs=1) as wpool,
            tc.tile_pool(name="state", bufs=2) as spool,
            tc.tile_pool(name="act", bufs=2) as apool,
            tc.tile_pool(name="psG", bufs=1, space="PSUM") as psG,
            tc.tile_pool(name="psT", bufs=1, space="PSUM") as psT,
            tc.tile_pool(name="psY", bufs=2, space="PSUM") as psY,
        ):
            WB_t = wpool.tile([128, NB], BF16, tag="WB")
            CF_t = wpool.tile([128, 1408], F32, tag="CF")
            nc.sync.dma_start(WB_t[:], wb[:])
            nc.sync.dma_start(CF_t[:], cf[:])

            IDT = CF_t[:, 0:128]
            BRZ = CF_t[:, 384:896]
            BNH = CF_t[:, 896:1152]
            BNI = CF_t[:, 1152:1408]

            # ---- initial state: Hp0|Hp1 = h_1 packed, hsb = h_1^T (bf16) ----
            Hp0 = spool.tile([128, 128], F32, tag="Hp0")
            Hp1 = spool.tile([128, 128], F32, tag="Hp1")
            nc.vector.tensor_copy(Hp0[:], CF_t[:, 128:256])
            nc.vector.tensor_copy(Hp1[:], CF_t[:, 256:384])

            def transpose_half(src_tile, tag_ps, tag_sb, engine):
                tp = psT.tile([128, 128], F32, tag=tag_ps)
                nc.tensor.transpose(tp[:], src_tile[:], IDT)
                h = spool.tile([128, 128], BF16, tag=tag_sb)
                if engine == "v":
                    nc.vector.tensor_copy(h[:], tp[:])
                else:
                    nc.scalar.activation(h[:], tp[:], AF.Copy)
                return h

            hsb0 = transpose_half(Hp0, "tp0", "hsb0", "v")
            hsb1 = transpose_half(Hp1, "tp1", "hsb1", "s")

            def y_block(h0, h1):
                """fc(h') from transposed state tiles -> psum tile."""
                yps = psY.tile([128, 192], F32, tag="yps")
                for k in range(KH):
                    src = h0 if k % 2 == 0 else h1
                    lhsT = src[:, 32 * (k // 2) : 32 * (k // 2) + 32]
                    for j in range(4):
                        nc.tensor.matmul(
                            yps[32 * j : 32 * j + 32, :],
                            lhsT,
                            wfy(k, j),
                            start=(k == 0),
                            stop=(k == KH - 1),
                            tile_position=(0, 32 * j),
                            skip_group_check=True,
                        )
                return yps

            for i in range(nsteps):
                # ---------- gate matmuls (consume hsb = h_{i+1}^T) ----------
                # three banks, streamed rz -> h_n -> i_n so that the sigmoids,
                # bias adds and rt = r*(h_n+bnh) hide behind the later passes.
                # The i_n half-split lets the half-0 chain (ns0/tanh0/h'0) run
                # while the second i_n half and the y matmuls still stream.
                grz = psG.tile([128, 512], F32, tag="grz")
                ghn = psG.tile([128, 256], F32, tag="ghn")
                gin = psG.tile([128, 256], F32, tag="gin")
                for base, lo, width, colo, gout, closes in (
                    (WRZ_OFF, 0, 512, 0, grz, True),
                    (WNIH_OFF, 256, 256, 0, ghn, True),
                    (WNIH_OFF, 0, 128, 0, gin, False),
                    (WNIH_OFF, 128, 128, 128, gin, True),
                ):
                    for k in KORDER:
                        src = hsb0 if k % 2 == 0 else hsb1
                        lhsT = src[:, 32 * (k // 2) : 32 * (k // 2) + 32]
                        first = k == KORDER[0]
                        last = k == KORDER[-1] and closes
                        for j in range(4):
                            off = base + (k * 4 + j) * 512 + lo
                            nc.tensor.matmul(
                                gout[32 * j : 32 * j + 32, colo : colo + width],
                                lhsT,
                                WB_t[:, off : off + width],
                                start=first,
                                stop=last,
                                tile_position=(0, 32 * j),
                                skip_group_check=True,
                            )

                phsb0, phsb1 = hsb0, hsb1  # y_i reads these after the transposes

                # ---------- elementwise: h_{i+2} = (1-z)*n + z*h_{i+1} ------
                # bias adds + sigmoids + zh + b1z hide behind the matmul
                # stream; zh/b1z run on the (otherwise idle) gpsimd engine
                trz = apool.tile([128, 512], F32, tag="trz")
                nc.vector.tensor_tensor(trz[:, 0:256], grz[:, 0:256], BRZ[:, 0:256], ALU.add)
                r = apool.tile([128, 256], F32, tag="r")
                nc.scalar.activation(r[:], trz[:, 0:256], AF.Sigmoid)
                nc.vector.tensor_tensor(trz[:, 256:512], grz[:, 256:512], BRZ[:, 256:512], ALU.add)
                z = apool.tile([128, 256], F32, tag="z")
                nc.scalar.activation(z[:], trz[:, 256:512], AF.Sigmoid)
                # b1z = 1 - z is affine, so it runs on ACT (idle between the
                # sigmoids and tanh); the zh halves go to the gpsimd engine
                b1z = apool.tile([128, 256], F32, tag="b1z")
                nc.scalar.activation(b1z[:], z[:], AF.Copy, bias=1.0, scale=-1.0)
                zh = apool.tile([128, 256], F32, tag="zh")
                nc.gpsimd.tensor_tensor(zh[:, 0:128], z[:, 0:128], Hp0[:], ALU.mult)
                nc.gpsimd.tensor_tensor(zh[:, 128:256], z[:, 128:256], Hp1[:], ALU.mult)

                # hnb/rt/rtb run while the i_n pass still streams
                hnb = apool.tile([128, 256], F32, tag="hnb")
                nc.vector.tensor_tensor(hnb[:], ghn[:], BNH[:], ALU.add)
                rt = apool.tile([128, 256], F32, tag="rt")
                nc.vector.tensor_tensor(rt[:], r[:], hnb[:], ALU.mult)
                rtb = apool.tile([128, 256], F32, tag="rtb")
                nc.vector.tensor_tensor(rtb[:], rt[:], BNI[:], ALU.add)

                # chain, half 0 first (feeds hsb0 -> next step's first chunks)
                ns = apool.tile([128, 256], F32, tag="ns")
                n = apool.tile([128, 256], F32, tag="n")
                Hp0n = spool.tile([128, 128], F32, tag="Hp0")
                Hp1n = spool.tile([128, 128], F32, tag="Hp1")
                for h0c, h1c, Hpn in ((0, 128, Hp0n), (128, 256, Hp1n)):
                    nc.vector.tensor_tensor(
                        ns[:, h0c:h1c], rtb[:, h0c:h1c], gin[:, h0c:h1c], ALU.add
                    )
                    nc.scalar.activation(n[:, h0c:h1c], ns[:, h0c:h1c], AF.Tanh)
                    nc.vector.tensor_tensor(
                        Hpn[:], n[:, h0c:h1c], b1z[:, h0c:h1c], ALU.mult
                    )
                    nc.vector.tensor_tensor(Hpn[:], Hpn[:], zh[:, h0c:h1c], ALU.add)
                Hp0, Hp1 = Hp0n, Hp1n

                # ---------- transposes for next step, then y matmuls --------
                # y (which only needs the OLD hsb) streams right after the
                # transposes, covering the PE wait for the hsb copies.
                hsb0 = transpose_half(Hp0, "tp0", "hsb0", "v")
                hsb1 = transpose_half(Hp1, "tp1", "hsb1", "s")
                if i > 0:
                    yps = y_block(phsb0, phsb1)
                    ysb = apool.tile([128, 192], F32, tag="ysb")
                    nc.scalar.activation(ysb[:], yps[:], AF.Copy)
                    nc.sync.dma_start(yo[i - 1], ysb[:])

            # ---------- drain final y (y_255 from h_256^T) ----------
            yps = y_block(hsb0, hsb1)
            ysb = apool.tile([128, 192], F32, tag="ysb")
            nc.scalar.activation(ysb[:], yps[:], AF.Copy)
            nc.sync.dma_start(yo[nsteps - 1], ysb[:])

    nc.compile()
    return nc


def _sigmoid(x):
    return 1.0 / (1.0 + np.exp(-x))


def _host_step0(src, hidden, w_ih, w_hh, b_ih, b_hh, w_fc, b_fc):
    """Reference step 0 in numpy f32: returns (h_1 [B,H], y_0 [B,O])."""
    x0 = src[0]
    h0 = hidden[0]
    gi = x0 @ w_ih.T + b_ih
    gh = h0 @ w_hh.T + b_hh
    r = _sigmoid(gi[:, 0:H] + gh[:, 0:H])
    z = _sigmoid(gi[:, H : 2 * H] + gh[:, H : 2 * H])
    n = np.tanh(gi[:, 2 * H :] + r * gh[:, 2 * H :])
    h1 = (1.0 - z) * n + z * h0
    y0 = h1 @ w_fc.T + b_fc
    return h1.astype(np.float32), y0.astype(np.float32)


def _prep_shared(w_ih, w_hh, b_ih, b_hh, w_fc, b_fc):
    """Host-side weight/bias packing shared by all cores -> (WB, CB)."""
    W_c = w_fc.T @ w_ih.T  # [H, 3H]
    Whh_T = np.ascontiguousarray(w_hh.T)  # [H, 3H]
    bc = b_ih + b_fc @ w_ih.T  # [3H]

    W_rz = W_c[:, : 2 * H] + Whh_T[:, : 2 * H]
    brz = bc[: 2 * H] + b_hh[: 2 * H]
    WNI = W_c[:, 2 * H :]
    WNH = Whh_T[:, 2 * H :]
    bni = bc[2 * H :]
    bnh = b_hh[2 * H :]

    wb = np.empty((128, NB), np.float32)
    for k in range(KH):
        rz_rows = W_rz[128 * k : 128 * k + 128]  # [128, 2048]
        ni_rows = WNI[128 * k : 128 * k + 128]  # [128, 1024]
        nh_rows = WNH[128 * k : 128 * k + 128]
        fy_rows = w_fc.T[128 * k : 128 * k + 128]  # [128, 768]
        for j in range(4):
            off = WRZ_OFF + (k * 4 + j) * 512
            wb[:, off : off + 256] = rz_rows[:, 256 * j : 256 * j + 256]
            wb[:, off + 256 : off + 512] = rz_rows[:, H + 256 * j : H + 256 * j + 256]
            off = WNIH_OFF + (k * 4 + j) * 512
            wb[:, off : off + 256] = ni_rows[:, 256 * j : 256 * j + 256]
            wb[:, off + 256 : off + 512] = nh_rows[:, 256 * j : 256 * j + 256]
            off = WFY_OFF + (k * 4 + j) * 192
            wb[:, off : off + 192] = fy_rows[:, 192 * j : 192 * j + 192]
    WB = wb.astype(ml_dtypes.bfloat16)

    def pack_bias(vec):  # [1024] -> [128, 256], rows 32j+b get slice j
        return np.repeat(vec.reshape(4, 256), 32, axis=0).astype(np.float32)

    biases = np.concatenate(
        [pack_bias(brz[:H]), pack_bias(brz[H:]), pack_bias(bnh), pack_bias(bni)],
        axis=1,
    )  # [128, 1024] = BRZ(512) | BNH(256) | BNI(256)
    return WB, biases


def _prep_core(h1, biases):
    """h1 [32, 1024] -> CF [128, 1408] f32 (IDT | HPI | BRZ | BNH | BNI)."""
    cfm = np.zeros((128, 1408), np.float32)
    cfm[:, 0:128] = np.eye(128, dtype=np.float32)
    cfm[:, 128:384] = h1.reshape(BC, 4, 256).transpose(1, 0, 2).reshape(128, 256)
    cfm[:, 384:1408] = biases
    return cfm


def kernel(src, tgt, hidden, w_ih, w_hh, b_ih, b_hh, w_fc, b_fc, **_kw):
    src = np.asarray(src, np.float32)
    hidden = np.asarray(hidden, np.float32)
    w_ih = np.asarray(w_ih, np.float32)
    w_hh = np.asarray(w_hh, np.float32)
    b_ih = np.asarray(b_ih, np.float32)
    b_hh = np.asarray(b_hh, np.float32)
    w_fc = np.asarray(w_fc, np.float32)
    b_fc = np.asarray(b_fc, np.float32)

    WB, biases = _prep_shared(w_ih, w_hh, b_ih, b_hh, w_fc, b_fc)
    h1, y0 = _host_step0(src, hidden, w_ih, w_hh, b_ih, b_hh, w_fc, b_fc)

    if NSTEPS not in _COMPILED:
        _COMPILED[NSTEPS] = _build_nc(NSTEPS)
    nc = _COMPILED[NSTEPS]

    in_maps = []
    for c in range(NCORES):
        sl = slice(BC * c, BC * (c + 1))
        in_maps.append(dict(WB=WB, CF=_prep_core(h1[sl], biases)))

    res = run_bass_kernel_spmd(nc, in_maps, list(range(NCORES)))

    out = np.empty((T, B, O), np.float32)
    out[0] = y0
    for c in range(NCORES):
        sl = slice(BC * c, BC * (c + 1))
        yo = np.asarray(res.results[c]["YO"])  # [255, 128, 192]
        out[1:, sl] = (
            yo.reshape(NSTEPS, 4, BC, 192).transpose(0, 2, 1, 3).reshape(NSTEPS, BC, O)
        )
    out[1:] += b_fc[None, None, :]
    return out
